# revision 7
# baseline (speedup 1.0000x reference)
import os
import numpy as np

B, T, C = 8, 1024, 768
H, HS = 12, 64
NPAIR = 6
NCK = 6
NT = 8

LAST_EXEC_NS = None
LAST_RESULTS = None

_cached_nc = None


def _round_f32r(a):
    u = a.view(np.uint32).astype(np.uint64)
    u = (u + 0x7FF + ((u >> 12) & 1)) & ~np.uint64(0xFFF)
    return u.astype(np.uint32).view(np.float32)


def _build_nc():
    import concourse.bass as bass
    import concourse.mybir as mybir
    import concourse.tile as tile
    from concourse import bacc
    from concourse.masks import make_upper_triangular

    f32 = mybir.dt.float32
    f32r = mybir.dt.float32r
    AF = mybir.ActivationFunctionType

    nc = bacc.Bacc("TRN2", target_bir_lowering=False, debug=False, num_devices=8)

    xT_d = nc.dram_tensor("xT", [C, T], f32r, kind="ExternalInput")
    wqk_d = nc.dram_tensor("wqk", [NPAIR, C, 256], f32r, kind="ExternalInput")
    wv_d = nc.dram_tensor("wv", [C, C], f32r, kind="ExternalInput")
    wp_d = nc.dram_tensor("wp", [H, HS, C], f32r, kind="ExternalInput")
    bp_d = nc.dram_tensor("bp", [C], f32, kind="ExternalInput")
    y_d = nc.dram_tensor("y", [T, C], f32, kind="ExternalOutput")

    with tile.TileContext(nc) as tc:
        with (
            tc.tile_pool(name="const", bufs=1) as const,
            tc.tile_pool(name="work", bufs=2) as work,
            tc.tile_pool(name="ppool", bufs=4) as ppool,
            tc.tile_pool(name="opool", bufs=1) as opool,
            tc.tile_pool(name="ps1", bufs=2, space="PSUM") as ps1,
            tc.tile_pool(name="ps2", bufs=1, space="PSUM") as ps2,
        ):
            xts = []
            for i in range(NCK):
                xt = const.tile([128, T], f32r, tag=f"xt{i}", name=f"xt{i}")
                nc.sync.dma_start(out=xt, in_=xT_d[i * 128:(i + 1) * 128, :])
                xts.append(xt)
            wvts = []
            for i in range(NCK):
                wvt = const.tile([128, C], f32r, tag=f"wv{i}", name=f"wv{i}")
                nc.sync.dma_start(out=wvt, in_=wv_d[i * 128:(i + 1) * 128, :])
                wvts.append(wvt)
            wpts = []
            for h in range(H):
                wpt = const.tile([HS, C], f32r, tag=f"wp{h}", name=f"wp{h}")
                nc.sync.dma_start(out=wpt, in_=wp_d[h])
                wpts.append(wpt)
            bias_t = const.tile([128, C], f32)
            nc.sync.dma_start(
                out=bias_t,
                in_=bass.AP(tensor=bp_d, offset=0, ap=[[0, 128], [1, C]]),
            )
            U = const.tile([128, 128], f32)
            make_upper_triangular(nc, U[:, :], val=1.0, diag=True)
            ones_f = const.tile([128, 64], f32)
            nc.vector.memset(ones_f, 1.0)
            ones_t = const.tile([128, 64], f32r)
            nc.vector.tensor_copy(out=ones_t, in_=ones_f)

            v_all = const.tile([128, NT, H * 65], f32r)
            v_heads = v_all.rearrange("p k (h c) -> p k h c", h=H)
            for kt in range(NT):
                nc.vector.tensor_copy(
                    out=v_heads[:, kt, :, 64:65],
                    in_=ones_f[:, 0:12].rearrange("p (h o) -> p h o", o=1),
                )

            outTs = [
                opool.tile([65, T], f32r, tag=f"outT{h}", name=f"outT{h}")
                for h in range(H)
            ]

            with nc.named_scope("vproj"):
                for kt in range(NT):
                    pv = ps2.tile([128, C], f32, tag="pv", bufs=1)
                    for ck in range(NCK):
                        for n0, n1 in ((0, 512), (512, 768)):
                            nc.tensor.matmul(
                                pv[:, n0:n1],
                                lhsT=xts[ck][:, kt * 128:(kt + 1) * 128],
                                rhs=wvts[ck][:, n0:n1],
                                start=(ck == 0),
                                stop=(ck == NCK - 1),
                            )
                    nc.scalar.copy(
                        out=v_heads[:, kt, :, 0:64],
                        in_=pv.rearrange("p (h c) -> p h c", h=H),
                    )

            for p in range(NPAIR):
                with nc.named_scope(f"qk{p}"):
                    wqkt = work.tile([128, NCK, 256], f32r, tag="wqkt", bufs=2)
                    nc.sync.dma_start(
                        out=wqkt,
                        in_=wqk_d[p].rearrange("(ck kp) m -> kp ck m", kp=128),
                    )
                    qT = work.tile([128, T], f32r, tag="qT", bufs=2)
                    kTt = work.tile([128, T], f32r, tag="kTt", bufs=2)
                    for dst, wo in ((qT, 0), (kTt, 128)):
                        for tch in range(2):
                            pqk = ps1.tile([128, 512], f32, tag="pscr", bufs=2)
                            for ck in range(NCK):
                                nc.tensor.matmul(
                                    pqk,
                                    lhsT=wqkt[:, ck, wo:wo + 128],
                                    rhs=xts[ck][:, tch * 512:(tch + 1) * 512],
                                    start=(ck == 0),
                                    stop=(ck == NCK - 1),
                                )
                            nc.vector.tensor_copy(
                                out=dst[:, tch * 512:(tch + 1) * 512], in_=pqk
                            )
                with nc.named_scope(f"att{p}"):
                    for qc in range(2):
                        nkt = 4 * (qc + 1)
                        po_pair = []
                        for hh in range(2):
                            po = ps1.tile([65, 512], f32, tag="po", bufs=4,
                                          name=f"po{hh}")
                            po_pair.append(po)
                        for kt in range(nkt):
                            col0 = max(0, 128 * kt - 512 * qc)
                            for hh in range(2):
                                h = 2 * p + hh
                                pscr = ps1.tile([128, 512], f32, tag="pscr",
                                                bufs=2, name="pscr")
                                nc.tensor.matmul(
                                    pscr[:, col0:512],
                                    lhsT=kTt[hh * 64:(hh + 1) * 64,
                                             kt * 128:(kt + 1) * 128],
                                    rhs=qT[hh * 64:(hh + 1) * 64,
                                           qc * 512 + col0:(qc + 1) * 512],
                                    start=True,
                                    stop=True,
                                )
                                pt = ppool.tile([128, 512], f32r, tag="pt",
                                                bufs=4, name="pt")
                                nc.scalar.activation(
                                    out=pt[:, col0:512],
                                    in_=pscr[:, col0:512],
                                    func=AF.Exp,
                                    scale=float(HS) ** -0.5,
                                )
                                if 128 * kt >= 512 * qc:
                                    nc.vector.tensor_mul(
                                        pt[:, col0:col0 + 128],
                                        pt[:, col0:col0 + 128],
                                        U,
                                    )
                                nc.tensor.matmul(
                                    po_pair[hh][:, col0:512],
                                    lhsT=v_all[:, kt, h * 65:(h + 1) * 65],
                                    rhs=pt[:, col0:512],
                                    start=(kt == 0),
                                    stop=(kt == nkt - 1),
                                )
                        for hh in range(2):
                            h = 2 * p + hh
                            qsl = slice(qc * 512, (qc + 1) * 512)
                            nc.vector.reciprocal(
                                out=po_pair[hh][64:65, :], in_=po_pair[hh][64:65, :]
                            )
                            nc.scalar.copy(out=outTs[h][:, qsl], in_=po_pair[hh])
                            pr = ps1.tile([64, 512], f32, tag="pscr", bufs=2,
                                          name="pr")
                            nc.tensor.matmul(
                                pr,
                                lhsT=ones_t[64:65, :],
                                rhs=outTs[h][64:65, qsl],
                                start=True,
                                stop=True,
                            )
                            nc.vector.tensor_mul(
                                outTs[h][0:64, qsl], outTs[h][0:64, qsl], pr
                            )

            with nc.named_scope("proj"):
                for tt in range(NT):
                    py = ps2.tile([128, C], f32, tag="pv", bufs=1, name="py")
                    for h in range(H):
                        for n0, n1 in ((0, 512), (512, 768)):
                            nc.tensor.matmul(
                                py[:, n0:n1],
                                lhsT=outTs[h][0:64, tt * 128:(tt + 1) * 128],
                                rhs=wpts[h][:, n0:n1],
                                start=(h == 0),
                                stop=(h == H - 1),
                            )
                    ysb = work.tile([128, C], f32, tag="ysb", bufs=3, name="ysb")
                    nc.vector.tensor_add(ysb, py, bias_t)
                    nc.sync.dma_start(
                        out=y_d[tt * 128:(tt + 1) * 128, :], in_=ysb
                    )

    nc.compile()
    return nc


def get_nc():
    global _cached_nc
    if _cached_nc is None:
        _cached_nc = _build_nc()
    return _cached_nc


def _host_pack(inputs):
    x = np.asarray(inputs["x"], dtype=np.float32)
    Wq = np.asarray(inputs["Wq"], dtype=np.float32)
    Wk = np.asarray(inputs["Wk"], dtype=np.float32)
    Wv = np.asarray(inputs["Wv"], dtype=np.float32)
    Wproj = np.asarray(inputs["Wproj"], dtype=np.float32)
    bproj = np.asarray(inputs["bproj"], dtype=np.float32)

    Wq2 = Wq.transpose(1, 0, 2).reshape(C, C)
    Wk2 = Wk.transpose(1, 0, 2).reshape(C, C)
    wqk = _round_f32r(
        np.ascontiguousarray(
            np.stack(
                [
                    np.concatenate(
                        [
                            Wq2[:, p * 128:(p + 1) * 128],
                            Wk2[:, p * 128:(p + 1) * 128],
                        ],
                        axis=1,
                    )
                    for p in range(NPAIR)
                ]
            )
        )
    )
    wv = _round_f32r(np.ascontiguousarray(Wv.transpose(1, 0, 2).reshape(C, C)))
    wp = _round_f32r(np.ascontiguousarray(Wproj.T.reshape(H, HS, C)))
    shared = {"wqk": wqk, "wv": wv, "wp": wp, "bp": bproj}
    in_maps = [
        dict(shared, xT=_round_f32r(np.ascontiguousarray(x[b].T)))
        for b in range(B)
    ]
    return in_maps


def kernel(**inputs):
    global LAST_EXEC_NS, LAST_RESULTS
    from concourse.bass_utils import run_bass_kernel_spmd

    nc = get_nc()
    in_maps = _host_pack(inputs)
    trace = bool(int(os.environ.get("KERNEL_TRACE", "0")))
    res = run_bass_kernel_spmd(
        nc, in_maps, core_ids=list(range(B)), trace=trace
    )
    LAST_EXEC_NS = res.exec_time_ns
    LAST_RESULTS = res
    y = np.stack([res.results[b]["y"] for b in range(B)])
    return y.astype(np.float32)


# revision 32
# speedup vs baseline: 2.0261x; 2.0261x over previous
import os
import numpy as np

B, T, C = 8, 1024, 768
H, HS = 12, 64
NPAIR = 6
NCK = 6
NT = 8

LAST_EXEC_NS = None
LAST_RESULTS = None

_cached_nc = None


def _round_f32r(a):
    u = a.view(np.uint32).astype(np.uint64)
    u = (u + 0x7FF + ((u >> 12) & 1)) & ~np.uint64(0xFFF)
    return u.astype(np.uint32).view(np.float32)


def _build_nc():
    import concourse.bass as bass
    import concourse.mybir as mybir
    import concourse.tile as tile
    from concourse import bacc
    from concourse.masks import make_upper_triangular

    f32 = mybir.dt.float32
    f32r = mybir.dt.float32r
    AF = mybir.ActivationFunctionType

    nc = bacc.Bacc("TRN2", target_bir_lowering=False, debug=False, num_devices=8)

    xT_d = nc.dram_tensor("xT", [C, T], f32r, kind="ExternalInput")
    wqk_d = nc.dram_tensor(
        "wqk", [NPAIR, 128, NCK, 256], f32r, kind="ExternalInput"
    )
    wv_d = nc.dram_tensor("wv", [C, C], f32r, kind="ExternalInput")
    wp_d = nc.dram_tensor("wp", [NPAIR, 128, C], f32r, kind="ExternalInput")
    bp_d = nc.dram_tensor("bp", [C], f32, kind="ExternalInput")
    y_d = nc.dram_tensor("y", [T, C], f32, kind="ExternalOutput")

    with tile.TileContext(nc) as tc:
        with (
            tc.tile_pool(name="const", bufs=1) as const,
            tc.tile_pool(name="work", bufs=2) as work,
            tc.tile_pool(name="ppool", bufs=4) as ppool,
            tc.tile_pool(name="opool", bufs=1) as opool,
            tc.tile_pool(name="ps1", bufs=2, space="PSUM") as ps1,
        ):
            wqkt0 = work.tile([128, NCK, 256], f32r, tag="wqkt", bufs=3,
                              name="wqkt")
            nc.gpsimd.dma_start(out=wqkt0, in_=wqk_d[0])
            xts = []
            for i in range(NCK):
                xt = const.tile([128, T], f32r, tag=f"xt{i}", name=f"xt{i}")
                eng = nc.sync if i % 2 == 0 else nc.scalar
                eng.dma_start(out=xt, in_=xT_d[i * 128:(i + 1) * 128, :])
                xts.append(xt)
            wvts = []
            for i in range(NCK):
                wvt = const.tile([128, C], f32r, tag=f"wv{i}", name=f"wv{i}")
                nc.scalar.dma_start(out=wvt, in_=wv_d[i * 128:(i + 1) * 128, :])
                wvts.append(wvt)
            U = const.tile([128, 128], f32)
            make_upper_triangular(nc, U[:, :], val=1.0, diag=True)
            ZU = const.tile([128, 256], f32)
            nc.vector.memset(ZU[:, 0:128], 0.0)
            nc.vector.tensor_copy(out=ZU[:, 128:256], in_=U)
            ones_f = const.tile([128, 12], f32)
            nc.vector.memset(ones_f, 1.0)
            ones_t = const.tile([128, 128], f32r)
            nc.vector.tensor_copy(
                out=ones_t, in_=ones_f[:, 0:1].broadcast_to([128, 128])
            )

            v_all = const.tile([128, NT, H * 65], f32r)
            v_heads = v_all.rearrange("p k (h c) -> p k h c", h=H)
            for kt in range(NT):
                nc.vector.tensor_copy(
                    out=v_heads[:, kt, :, 64:65],
                    in_=ones_f.rearrange("p (h o) -> p h o", o=1),
                )

            outTs = [
                opool.tile([128, T], f32r, tag=f"outT{p}", name=f"outT{p}")
                for p in range(NPAIR)
            ]
            rs3 = [
                opool.tile([128, T], f32r, tag=f"rs{j}", name=f"rs{j}")
                for j in range(3)
            ]

            def emit_vproj(kts):
                with nc.named_scope("vproj"):
                    for kt in kts:
                        pv = ps1.tile([128, C], f32, tag="big2", bufs=2,
                                      name="pv")
                        for ck in range(NCK):
                            for n0, n1 in ((0, 512), (512, 768)):
                                nc.tensor.matmul(
                                    pv[:, n0:n1],
                                    lhsT=xts[ck][:, kt * 128:(kt + 1) * 128],
                                    rhs=wvts[ck][:, n0:n1],
                                    start=(ck == 0),
                                    stop=(ck == NCK - 1),
                                )
                        nc.scalar.copy(
                            out=v_heads[:, kt, :, 0:64],
                            in_=pv.rearrange("p (h c) -> p h c", h=H),
                        )

            qkts = {}

            def emit_qkT(p, wqkt=None):
                with nc.named_scope(f"qk{p}"):
                    if wqkt is None:
                        wqkt = work.tile([128, NCK, 256], f32r, tag="wqkt",
                                         bufs=3, name="wqkt")
                        nc.sync.dma_start(out=wqkt, in_=wqk_d[p])
                    qT = work.tile([128, T], f32r, tag="qT", bufs=3, name="qT")
                    kTt = work.tile([128, T], f32r, tag="kTt", bufs=3,
                                    name="kTt")
                    for dst, wo in ((qT, 0), (kTt, 128)):
                        for tch in range(2):
                            pqk = ps1.tile([128, 512], f32, tag="po", bufs=4,
                                           name="pqk")
                            for ck in range(NCK):
                                nc.tensor.matmul(
                                    pqk,
                                    lhsT=wqkt[:, ck, wo:wo + 128],
                                    rhs=xts[ck][:, tch * 512:(tch + 1) * 512],
                                    start=(ck == 0),
                                    stop=(ck == NCK - 1),
                                )
                            nc.scalar.copy(
                                out=dst[:, tch * 512:(tch + 1) * 512], in_=pqk
                            )
                    qkts[p] = (qT, kTt)

            emit_qkT(0, wqkt0)
            emit_vproj(range(0, 4))

            for p in range(NPAIR):
                qT, kTt = qkts.pop(p)
                with nc.named_scope(f"att{p}"):
                    for qc in range(2):
                        if qc == 1 and p + 1 < NPAIR:
                            emit_qkT(p + 1)
                        if qc == 1 and p == 0:
                            emit_vproj(range(4, NT))
                        nkt = 4 * (qc + 1)
                        po_pair = []
                        for hh in range(2):
                            po = ps1.tile([65, 512], f32, tag="po", bufs=4,
                                          name=f"po{hh}")
                            po_pair.append(po)
                        for kt in range(nkt):
                            col0 = max(0, 128 * kt - 512 * qc)
                            ccol = 256 if col0 == 384 else col0
                            pt = ppool.tile([128, 2, 512], f32r, tag="pt",
                                            bufs=6, name="pt")
                            pscr = ps1.tile([128, 2, 512], f32, tag="big2",
                                            bufs=2, name="pscr")
                            for hh in range(2):
                                nc.tensor.matmul(
                                    pscr[:, hh, ccol:512],
                                    lhsT=kTt[hh * 64:(hh + 1) * 64,
                                             kt * 128:(kt + 1) * 128],
                                    rhs=qT[hh * 64:(hh + 1) * 64,
                                           qc * 512 + ccol:(qc + 1) * 512],
                                    start=True,
                                    stop=True,
                                )
                            nc.scalar.activation(
                                out=pt[:, :, ccol:512],
                                in_=pscr[:, :, ccol:512],
                                func=AF.Exp,
                                scale=float(HS) ** -0.5,
                            )
                            if 128 * kt >= 512 * qc:
                                mask = ZU if col0 == 384 else U
                                mw = 512 - ccol if col0 == 384 else 128
                                nc.gpsimd.tensor_mul(
                                    pt[:, :, ccol:ccol + mw],
                                    pt[:, :, ccol:ccol + mw],
                                    mask.rearrange("p (o c) -> p o c", o=1)
                                        .broadcast_to([128, 2, mw]),
                                )
                            for hh in range(2):
                                h = 2 * p + hh
                                nc.tensor.matmul(
                                    po_pair[hh][:, ccol:512],
                                    lhsT=v_all[:, kt, h * 65:(h + 1) * 65],
                                    rhs=pt[:, hh, ccol:512],
                                    start=(kt == 0),
                                    stop=(kt == nkt - 1),
                                )
                        qsl = slice(qc * 512, (qc + 1) * 512)
                        for hh in range(2):
                            h = 2 * p + hh
                            rrow = rs3[h // 4][(h % 4) * 32:(h % 4) * 32 + 1,
                                               qsl]
                            with nc.allow_low_precision(
                                reason="1/rowsum at fp32r (12-bit mantissa) "
                                       "costs ~1e-4 relative error"
                            ):
                                nc.vector.reciprocal(
                                    out=rrow, in_=po_pair[hh][64:65, :]
                                )
                            nc.vector.tensor_copy(
                                out=outTs[p][hh * 64:(hh + 1) * 64, qsl],
                                in_=po_pair[hh][0:64, :],
                            )
                        for hh in range(2):
                            h = 2 * p + hh
                            r0 = (h % 4) * 32
                            pr = ps1.tile([128, 512], f32, tag="po", bufs=4,
                                          name="pr")
                            nc.tensor.matmul(
                                pr,
                                lhsT=ones_t[r0:r0 + 1, :],
                                rhs=rs3[h // 4][r0:r0 + 1, qsl],
                                start=True,
                                stop=True,
                                tile_position=(r0, 0),
                            )
                            nc.vector.tensor_mul(
                                outTs[p][hh * 64:(hh + 1) * 64, qsl],
                                outTs[p][hh * 64:(hh + 1) * 64, qsl],
                                pr[hh * 64:(hh + 1) * 64, :],
                            )

            wpts = []
            for pp in range(NPAIR):
                wpt = const.tile([128, C], f32r, tag=f"wp{pp}", name=f"wp{pp}")
                nc.gpsimd.dma_start(out=wpt, in_=wp_d[pp])
                wpts.append(wpt)
            bias_t = const.tile([128, C], f32)
            nc.gpsimd.dma_start(
                out=bias_t,
                in_=bass.AP(tensor=bp_d, offset=0, ap=[[0, 128], [1, C]]),
            )

            with nc.named_scope("proj"):
                for tt in range(NT):
                    py = ps1.tile([128, C], f32, tag="big2", bufs=2, name="py")
                    for p in range(NPAIR):
                        for n0, n1 in ((0, 512), (512, 768)):
                            nc.tensor.matmul(
                                py[:, n0:n1],
                                lhsT=outTs[p][:, tt * 128:(tt + 1) * 128],
                                rhs=wpts[p][:, n0:n1],
                                start=(p == 0),
                                stop=(p == NPAIR - 1),
                            )
                    ysb = work.tile([128, C], f32, tag="ysb", bufs=3,
                                    name="ysb")
                    nc.vector.tensor_add(ysb, py, bias_t)
                    nc.sync.dma_start(
                        out=y_d[tt * 128:(tt + 1) * 128, :], in_=ysb
                    )

    nc.compile()
    return nc


def get_nc():
    global _cached_nc
    if _cached_nc is None:
        _cached_nc = _build_nc()
    return _cached_nc


def _host_pack(inputs):
    x = np.asarray(inputs["x"], dtype=np.float32)
    Wq = np.asarray(inputs["Wq"], dtype=np.float32)
    Wk = np.asarray(inputs["Wk"], dtype=np.float32)
    Wv = np.asarray(inputs["Wv"], dtype=np.float32)
    Wproj = np.asarray(inputs["Wproj"], dtype=np.float32)
    bproj = np.asarray(inputs["bproj"], dtype=np.float32)

    Wq2 = Wq.transpose(1, 0, 2).reshape(C, C)
    Wk2 = Wk.transpose(1, 0, 2).reshape(C, C)
    wqk = np.stack(
        [
            np.concatenate(
                [
                    Wq2[:, p * 128:(p + 1) * 128],
                    Wk2[:, p * 128:(p + 1) * 128],
                ],
                axis=1,
            )
            for p in range(NPAIR)
        ]
    )
    wqk = _round_f32r(
        np.ascontiguousarray(
            wqk.reshape(NPAIR, NCK, 128, 256).transpose(0, 2, 1, 3)
        )
    )
    wv = _round_f32r(np.ascontiguousarray(Wv.transpose(1, 0, 2).reshape(C, C)))
    wp = _round_f32r(np.ascontiguousarray(Wproj.T.reshape(NPAIR, 128, C)))
    shared = {"wqk": wqk, "wv": wv, "wp": wp, "bp": bproj}
    in_maps = [
        dict(shared, xT=_round_f32r(np.ascontiguousarray(x[b].T)))
        for b in range(B)
    ]
    return in_maps


def kernel(**inputs):
    global LAST_EXEC_NS, LAST_RESULTS
    from concourse.bass_utils import run_bass_kernel_spmd

    nc = get_nc()
    in_maps = _host_pack(inputs)
    trace = bool(int(os.environ.get("KERNEL_TRACE", "0")))
    res = run_bass_kernel_spmd(
        nc, in_maps, core_ids=list(range(B)), trace=trace
    )
    LAST_EXEC_NS = res.exec_time_ns
    LAST_RESULTS = res
    y = np.stack([res.results[b]["y"] for b in range(B)])
    return y.astype(np.float32)
